# revision 10
# baseline (speedup 1.0000x reference)
"""Distributed Trainium2 Bass kernel for nn_GCNPredictor (3-layer GCN + MLP heads).

Contract: kernel(**inputs) takes the FULL unsharded inputs and returns the
FULL [2T, 1] float32 output. Internally shards nodes across 8 NeuronCores.

Algorithm (mathematically identical to the PyG-style reference):
    deg   = segment_sum(ew, dst) + 1 ;  dinv = rsqrt(deg)
    per GCN layer l:  table t = dinv * (h @ W_l)          [row-major, bf16]
                      agg[d]  = sum_e c_e * t[src_e]      (c_e = dinv[dst]*ew;
                                                           self-edge c = dinv[d])
                      h_next  = relu(agg + b_l)
    head: h4 = relu(h3 @ Wh + bh); ace/h2 = h4 @ Wace/Wh2 + biases

Device mapping per core (rows sharded, 6272 rows = 49 tiles of 128):
    - AllGather of the bf16 table each layer (HBM->HBM collective)
    - dma_gather edge-major chunks of 128 edges (int16 idx; two overlapping
      table views A=[0:32768] / B=[17408:50176] beat the int16 range limit)
    - selector matrices sel[e, d] = (iota_d == dstloc_e) * c_e built on DVE
      with step-0 broadcast APs, one is_equal + one in-place mult per group
    - PE matmul psum[feat, dst] += gathered_chunk.T @ sel_chunk (PSUM f32)
    - ACT evicts with fused bias+relu; next table via PE matmul with W;
      dinv fold + bf16 cast fused into the ACT eviction
"""

import sys

for _p in ("/opt/trn_rl_repo", "/opt/pypackages"):
    if _p not in sys.path:
        sys.path.insert(0, _p)

import numpy as np
import ml_dtypes

import concourse.bass as bass
import concourse.mybir as mybir
import concourse.bacc as bacc
import concourse.tile as tile
from concourse import bass_utils
from concourse.masks import make_identity

BF16 = ml_dtypes.bfloat16

# ---- problem constants (hardcoded per contract) ----
N = 50000
E = 640000
D = 128
T = 100
NCORES = 8
P = 128
NT = 49                  # dst tiles per core
RPC = NT * P             # 6272 rows per core
NPAD = NCORES * RPC      # 50176 padded rows
A_ROWS = 32768           # table view A = rows [0, 32768)
B_BASE = NPAD - 32768    # 17408; view B = rows [17408, 50176)
GROUP_TILES = 5          # dst tiles per gather group
NGROUPS = (NT + GROUP_TILES - 1) // GROUP_TILES

_program_cache = {}
DEBUG_STAGE = 0  # 0=full; 1..5 truncate program for HW fault bisect


# ----------------------------------------------------------------------------
# Host-side planning: shard edges, split per (core, tile, half), pad to
# cross-core-common chunk counts, build gather index / selector-coef arrays.
# ----------------------------------------------------------------------------
def _plan(edge_index, edge_weight):
    src = edge_index[0].astype(np.int64)
    dst = edge_index[1].astype(np.int64)
    ew = edge_weight.astype(np.float32)

    deg = np.bincount(dst, weights=ew.astype(np.float64), minlength=N).astype(
        np.float32
    ) + 1.0
    dinv = (1.0 / np.sqrt(np.maximum(deg, 1e-12))).astype(np.float32)

    # real edges only; self-loop term is applied on-device via a scaled
    # PE transpose-accumulate of the core's own table rows
    all_src = src
    all_dst = dst
    all_c = dinv[dst] * ew

    core = all_dst // RPC
    tl = (all_dst % RPC) // P
    dstloc = (all_dst % P).astype(np.float32)

    # sort by (core, tile, src): within a bucket the A-only prefix
    # (src < B_BASE) comes first, then flexible, then B-only (src >= A_ROWS)
    order = np.lexsort((all_src, tl, core))
    s_src = all_src[order]
    s_c = all_c[order]
    s_dl = dstloc[order]
    s_core = core[order]
    s_tl = tl[order]

    # bucket boundaries for all 8*49 (core,tile) buckets
    key = s_core * NT + s_tl
    bounds = np.searchsorted(key, np.arange(NCORES * NT + 1))

    # per-bucket counts
    tot = np.diff(bounds).reshape(NCORES, NT)
    lowA = np.zeros((NCORES, NT), np.int64)
    flex = np.zeros((NCORES, NT), np.int64)
    for b in range(NCORES * NT):
        lo, hi = bounds[b], bounds[b + 1]
        ss = s_src[lo:hi]
        lowA[b // NT, b % NT] = np.searchsorted(ss, B_BASE)
        flex[b // NT, b % NT] = np.searchsorted(ss, A_ROWS) - lowA[b // NT, b % NT]

    # common per-tile chunk counts nA[t], nB[t]
    targetA = np.clip(tot // 2, lowA, lowA + flex)
    nA = np.maximum.reduce(-(-targetA // P), axis=0)  # ceil, max over cores
    takeA = np.minimum(nA[None, :] * P, lowA + flex)
    nB = np.maximum.reduce(-(-(tot - takeA) // P), axis=0)

    K_tile = nA + nB
    # group layout: for each group, A-chunks of its tiles then B-chunks
    gidx = []  # per group: (kA0, nAg, kB0, nBg)
    tmeta = []  # per tile: (group, a_off, a_cnt, b_off, b_cnt) offsets in group buf
    k = 0
    for g in range(NGROUPS):
        ts = range(g * GROUP_TILES, min((g + 1) * GROUP_TILES, NT))
        nAg = int(sum(nA[t] for t in ts))
        nBg = int(sum(nB[t] for t in ts))
        kA0, kB0 = k, k + nAg
        ao, bo = 0, nAg
        for t in ts:
            tmeta.append((g, ao, int(nA[t]), bo, int(nB[t])))
            ao += int(nA[t])
            bo += int(nB[t])
        gidx.append((kA0, nAg, kB0, nBg))
        k += nAg + nBg
    K_tot = k

    # fill slot arrays per core
    idx_slots = np.zeros((NCORES, K_tot * P), np.int16)
    c_slots = np.zeros((NCORES, K_tot * P), np.float32)
    dl_slots = np.zeros((NCORES, K_tot * P), np.float32)
    for cix in range(NCORES):
        for t in range(NT):
            b = cix * NT + t
            lo, hi = bounds[b], bounds[b + 1]
            ta = int(takeA[cix, t])
            g, ao, ac, bo, bc = tmeta[t]
            kA0, nAg, kB0, nBg = gidx[g]
            # A half
            a0 = (kA0 + ao) * P
            idx_slots[cix, a0 : a0 + ta] = s_src[lo : lo + ta]
            c_slots[cix, a0 : a0 + ta] = s_c[lo : lo + ta]
            dl_slots[cix, a0 : a0 + ta] = s_dl[lo : lo + ta]
            # B half (remaining edges; idx relative to B_BASE)
            nb_real = hi - lo - ta
            b0 = (kB0 + (bo - nAg)) * P
            idx_slots[cix, b0 : b0 + nb_real] = s_src[lo + ta : hi] - B_BASE
            c_slots[cix, b0 : b0 + nb_real] = s_c[lo + ta : hi]
            dl_slots[cix, b0 : b0 + nb_real] = s_dl[lo + ta : hi]

    # wrap idx into the dma_gather layout: slot i -> [i % 16, i // 16],
    # replicated across the 8 q7 cores (partitions 16..127)
    idx_wrapped = np.empty((NCORES, 128, K_tot * 8), np.int16)
    dl_arr = np.empty((NCORES, 128, K_tot), BF16)
    c_arr = np.empty((NCORES, 128, K_tot), BF16)
    for cix in range(NCORES):
        w = idx_slots[cix].reshape(K_tot * 8, 16).T  # [16, K*8]
        idx_wrapped[cix] = np.tile(w, (8, 1))
        dl_arr[cix] = dl_slots[cix].reshape(K_tot, P).T.astype(BF16)
        c_arr[cix] = c_slots[cix].reshape(K_tot, P).T.astype(BF16)

    # per-core dinv [128, NT] (pad rows -> 0 so pad table rows are zeroed)
    dinv_pad = np.zeros(NPAD, np.float32)
    dinv_pad[:N] = dinv
    dinv_arr = dinv_pad.reshape(NCORES, NT, P).transpose(0, 2, 1).copy()

    return dict(
        gidx=gidx,
        tmeta=tmeta,
        K_tot=K_tot,
        idx=idx_wrapped,
        dl=dl_arr,
        c=c_arr,
        dinv=dinv_arr,
    )


# ----------------------------------------------------------------------------
# Bass program build (SPMD; per-core differences live only in input data)
# ----------------------------------------------------------------------------
def _build_program(gidx, tmeta, K_tot):
    bf16 = mybir.dt.bfloat16
    f32 = mybir.dt.float32

    nc = bacc.Bacc(
        "TRN2", target_bir_lowering=False, debug=False, num_devices=NCORES,
        num_swdge_queues=4,
    )

    x_d = nc.dram_tensor("x", [RPC, P], f32, kind="ExternalInput")
    idx_d = nc.dram_tensor("idx", [128, K_tot * 8], mybir.dt.int16, kind="ExternalInput")
    dl_d = nc.dram_tensor("dl", [128, K_tot], bf16, kind="ExternalInput")
    c_d = nc.dram_tensor("c", [128, K_tot], bf16, kind="ExternalInput")
    dinv_d = nc.dram_tensor("dinv", [128, NT], f32, kind="ExternalInput")
    w_d = [
        nc.dram_tensor(f"w{i}", [P, P], bf16, kind="ExternalInput") for i in range(4)
    ]
    whead_d = nc.dram_tensor("whead", [P, 2], bf16, kind="ExternalInput")
    b_d = [
        nc.dram_tensor(f"b{i}", [P, 1], f32, kind="ExternalInput") for i in range(4)
    ]
    bhead_d = nc.dram_tensor("bhead", [2, 1], f32, kind="ExternalInput")
    out_d = nc.dram_tensor("out", [2, RPC], f32, kind="ExternalOutput")

    with tile.TileContext(nc) as tc:
        with (
            tc.tile_pool(name="const", bufs=1) as cpool,
            tc.tile_pool(name="stage", bufs=2) as stpool,
            tc.tile_pool(name="gather", bufs=2) as gpool,
            tc.tile_pool(name="sel", bufs=2) as spool,
            tc.tile_pool(name="hT", bufs=3) as hpool,
            tc.tile_pool(name="xp", bufs=3) as xpool,
            tc.tile_pool(name="agg_ps", bufs=3, space="PSUM") as aggps,
            tc.tile_pool(name="mm_ps", bufs=2, space="PSUM") as mmps,
            tc.tile_pool(name="hd_ps", bufs=1, space="PSUM") as hdps,
            tc.tile_pool(name="dram", bufs=1, space="DRAM") as dpool,
        ):
            # ---- resident constants ----
            idx_sb = cpool.tile([128, K_tot * 8], mybir.dt.int16)
            dl_sb = cpool.tile([128, K_tot], bf16)
            c_sb = cpool.tile([128, K_tot], bf16)
            dinv_sb = cpool.tile([128, NT], f32)
            w_sb = [cpool.tile([P, P], bf16, tag=f"w{i}", name=f"w{i}_sb") for i in range(4)]
            whead_sb = cpool.tile([P, 2], bf16)
            b_sb = [cpool.tile([P, 1], f32, tag=f"b{i}", name=f"b{i}_sb") for i in range(4)]
            bhead_sb = cpool.tile([2, 1], f32)
            iota_sb = cpool.tile([P, P], bf16)
            ident_sb = cpool.tile([P, P], bf16)
            identf_sb = cpool.tile([P, P], f32)

            nc.sync.dma_start(out=idx_sb[:], in_=idx_d[:])
            nc.sync.dma_start(out=dl_sb[:], in_=dl_d[:])
            nc.sync.dma_start(out=c_sb[:], in_=c_d[:])
            nc.sync.dma_start(out=dinv_sb[:], in_=dinv_d[:])
            for i in range(4):
                nc.sync.dma_start(out=w_sb[i][:], in_=w_d[i][:])
                nc.sync.dma_start(out=b_sb[i][:], in_=b_d[i][:])
            nc.sync.dma_start(out=whead_sb[:], in_=whead_d[:])
            nc.sync.dma_start(out=bhead_sb[:], in_=bhead_d[:])
            nc.gpsimd.iota(
                iota_sb[:], pattern=[[1, P]], base=0, channel_multiplier=0,
                allow_small_or_imprecise_dtypes=True,
            )
            make_identity(nc, ident_sb[:])
            make_identity(nc, identf_sb[:])

            # AG buffers (table per layer)
            ag_in = [dpool.tile([RPC, P], bf16, tag=f"agin{l}", name=f"agin{l}") for l in range(3)]
            ag_out = [
                dpool.tile([NPAD, P], bf16, addr_space="Shared", tag=f"agout{l}",
                           name=f"agout{l}")
                for l in range(3)
            ]

            def stage_to_dram(stage_sb, dram_ap):
                # stage [128, NT*128] (row r = t*128+p at [p, t*128+f]) -> [RPC, P]
                nc.sync.dma_start(
                    out=dram_ap.rearrange("(t p) f -> p t f", p=P),
                    in_=stage_sb.rearrange("p (t f) -> p t f", f=P),
                )

            # ---- layer-1 table: t1 = dinv * (x @ W1), row-major bf16 ----
            tstage = stpool.tile([128, NT * P], bf16, tag="tstage")
            for t in range(NT):
                x_bf = xpool.tile([P, P], bf16, tag="xbf")
                nc.gpsimd.dma_start(
                    out=x_bf[:], in_=x_d[t * P : (t + 1) * P, :]
                )  # f32 -> bf16 cast during DMA
                xT_ps = mmps.tile([P, P], bf16, tag="xT")
                nc.tensor.transpose(out=xT_ps[:], in_=x_bf[:], identity=ident_sb[:])
                xT_sb = xpool.tile([P, P], bf16, tag="xT_sb")
                nc.vector.tensor_copy(out=xT_sb[:], in_=xT_ps[:])
                xw_ps = mmps.tile([P, P], f32, tag="xw")
                nc.tensor.matmul(
                    out=xw_ps[:], lhsT=xT_sb[:], rhs=w_sb[0][:], start=True, stop=True
                )
                nc.scalar.activation(
                    out=tstage[:, t * P : (t + 1) * P],
                    in_=xw_ps[:],
                    func=mybir.ActivationFunctionType.Copy,
                    scale=dinv_sb[:, t : t + 1],
                )
            stage_to_dram(tstage[:], ag_in[0][:])
            nc.gpsimd.collective_compute(
                "AllGather",
                mybir.AluOpType.bypass,
                replica_groups=[list(range(NCORES))],
                ins=[ag_in[0][:]],
                outs=[ag_out[0][:]],
            )

            # ---- output staging ----
            outstage = cpool.tile([2, RPC], f32)
            if DEBUG_STAGE:
                nc.gpsimd.memset(outstage[:], 0.0)

            # ---- 3 GCN layers ----
            tstage_prev = tstage
            for l in range(3):
                tab = ag_out[l]
                tabA = tab[0:A_ROWS, :]
                tabB = tab[B_BASE : B_BASE + A_ROWS, :]
                if l < 2 and DEBUG_STAGE not in (2, 3):
                    tstage2 = stpool.tile([128, NT * P], bf16, tag="tstage")

                if DEBUG_STAGE == 1:
                    break
                for g, (kA0, nAg, kB0, nBg) in enumerate(gidx):
                    Cg = nAg + nBg
                    k0 = kA0  # global chunk range [k0, k0+Cg) is contiguous
                    gbuf = gpool.tile([128, Cg, P], bf16, tag="gbuf")
                    q_a = (l * len(gidx) * 2 + 2 * g) % 4
                    q_b = (l * len(gidx) * 2 + 2 * g + 1) % 4
                    if nAg > 0:
                        nc.gpsimd.dma_gather(
                            gbuf[:, 0:nAg, :],
                            tabA,
                            idx_sb[:, kA0 * 8 : (kA0 + nAg) * 8],
                            nAg * P,
                            nAg * P,
                            P,
                            elem_step=tabA.ap[0][0],
                            single_packet=False,
                            queue_num=q_a,
                        )
                    if nBg > 0:
                        nc.gpsimd.dma_gather(
                            gbuf[:, nAg:Cg, :],
                            tabB,
                            idx_sb[:, kB0 * 8 : (kB0 + nBg) * 8],
                            nBg * P,
                            nBg * P,
                            P,
                            elem_step=tabB.ap[0][0],
                            single_packet=False,
                            queue_num=q_b,
                        )

                    if DEBUG_STAGE == 2:
                        probe = hpool.tile([2, P], f32, tag="probe")
                        nc.vector.tensor_copy(out=probe[:], in_=gbuf[0:2, 0, :])
                        nc.vector.tensor_copy(
                            out=outstage[:, g * P : (g + 1) * P], in_=probe[:]
                        )
                        continue
                    sel = spool.tile([128, Cg, P], bf16, tag="sel")
                    nc.vector.tensor_tensor(
                        out=sel[:],
                        in0=iota_sb[:]
                        .rearrange("p (g d) -> p g d", g=1)
                        .to_broadcast([128, Cg, P]),
                        in1=dl_sb[:, k0 : k0 + Cg]
                        .rearrange("p (g o) -> p g o", o=1)
                        .to_broadcast([128, Cg, P]),
                        op=mybir.AluOpType.is_equal,
                    )
                    nc.vector.tensor_tensor(
                        out=sel[:],
                        in0=sel[:],
                        in1=c_sb[:, k0 : k0 + Cg]
                        .rearrange("p (g o) -> p g o", o=1)
                        .to_broadcast([128, Cg, P]),
                        op=mybir.AluOpType.mult,
                    )

                    if DEBUG_STAGE == 3:
                        probe = hpool.tile([2, P], f32, tag="probe")
                        nc.vector.tensor_copy(out=probe[:], in_=sel[0:2, 0, :])
                        nc.vector.tensor_copy(
                            out=outstage[:, g * P : (g + 1) * P], in_=probe[:]
                        )
                        continue
                    for t in range(g * GROUP_TILES, min((g + 1) * GROUP_TILES, NT)):
                        _, ao, ac, bo, bc = tmeta[t]
                        chunks = list(range(ao, ao + ac)) + list(range(bo, bo + bc))
                        agg = aggps.tile([P, P], f32, tag="agg")
                        # self-loop: agg[feat, d] += dinv[d] * t_own[d, feat]
                        # (scaled rows transpose-accumulated through the PE)
                        tsrc = tstage if l == 0 else tstage_prev
                        selfsc = xpool.tile([P, P], f32, tag="selfsc")
                        nc.scalar.activation(
                            out=selfsc[:],
                            in_=tsrc[:, t * P : (t + 1) * P],
                            func=mybir.ActivationFunctionType.Copy,
                            scale=dinv_sb[:, t : t + 1],
                        )
                        nc.tensor.matmul(
                            out=agg[:],
                            lhsT=selfsc[:],
                            rhs=identf_sb[:],
                            is_transpose=True,
                            start=True,
                            stop=(len(chunks) == 0),
                        )
                        for j, ch in enumerate(chunks):
                            nc.tensor.matmul(
                                out=agg[:],
                                lhsT=gbuf[:, ch, :],
                                rhs=sel[:, ch, :],
                                start=False,
                                stop=(j == len(chunks) - 1),
                            )
                        # h_lT[feat, dst] = relu(agg + b_l)
                        hT = hpool.tile([P, P], bf16, tag="hT")
                        nc.scalar.activation(
                            out=hT[:],
                            in_=agg[:],
                            func=mybir.ActivationFunctionType.Relu,
                            bias=b_sb[l][:],
                            scale=1.0,
                        )
                        if DEBUG_STAGE == 35:
                            nc.vector.tensor_copy(
                                out=outstage[:, t * P : (t + 1) * P], in_=hT[0:2, :]
                            )
                            continue
                        if l < 2:
                            # next table rows: t_next = dinv * (h @ W_{l+1})
                            tw_ps = mmps.tile([P, P], f32, tag="xw")
                            nc.tensor.matmul(
                                out=tw_ps[:],
                                lhsT=hT[:],
                                rhs=w_sb[l + 1][:],
                                start=True,
                                stop=True,
                            )
                            nc.scalar.activation(
                                out=tstage2[:, t * P : (t + 1) * P],
                                in_=tw_ps[:],
                                func=mybir.ActivationFunctionType.Copy,
                                scale=dinv_sb[:, t : t + 1],
                            )
                        else:
                            # h4T = relu(Wh.T-form + bh); heads = Wboth.T @ h4T
                            h4_ps = mmps.tile([P, P], f32, tag="xw")
                            nc.tensor.matmul(
                                out=h4_ps[:],
                                lhsT=w_sb[3][:],
                                rhs=hT[:],
                                start=True,
                                stop=True,
                            )
                            h4T = hpool.tile([P, P], bf16, tag="h4T")
                            nc.scalar.activation(
                                out=h4T[:],
                                in_=h4_ps[:],
                                func=mybir.ActivationFunctionType.Relu,
                                bias=b_sb[3][:],
                                scale=1.0,
                            )
                            hd_ps = hdps.tile([2, P], f32, tag="hd")
                            nc.tensor.matmul(
                                out=hd_ps[:],
                                lhsT=whead_sb[:],
                                rhs=h4T[:],
                                start=True,
                                stop=True,
                            )
                            nc.scalar.activation(
                                out=outstage[:, t * P : (t + 1) * P],
                                in_=hd_ps[:],
                                func=mybir.ActivationFunctionType.Identity,
                                bias=bhead_sb[:],
                                scale=1.0,
                            )

                if DEBUG_STAGE in (2, 3, 4):
                    break
                if l < 2:
                    stage_to_dram(tstage2[:], ag_in[l + 1][:])
                    tstage_prev = tstage2
                    nc.gpsimd.collective_compute(
                        "AllGather",
                        mybir.AluOpType.bypass,
                        replica_groups=[list(range(NCORES))],
                        ins=[ag_in[l + 1][:]],
                        outs=[ag_out[l + 1][:]],
                    )

            nc.sync.dma_start(out=out_d[:], in_=outstage[:])

    nc.compile()
    return nc


# ----------------------------------------------------------------------------
# Entry point
# ----------------------------------------------------------------------------
def kernel(
    x, edge_index, edge_weight, ace_idx, h2_idx,
    W1, b1, W2, b2, W3, b3, Wh, bh, Wace, bace, Wh2, bh2,
    _return_exec_info=False,
):
    x = np.asarray(x, np.float32)
    edge_index = np.asarray(edge_index, np.int32)
    edge_weight = np.asarray(edge_weight, np.float32)
    plan = _plan(edge_index, edge_weight)

    key = (plan["K_tot"], tuple(plan["tmeta"]), tuple(plan["gidx"]))
    if key not in _program_cache:
        _program_cache[key] = _build_program(
            plan["gidx"], plan["tmeta"], plan["K_tot"]
        )
    nc = _program_cache[key]

    x_pad = np.zeros((NPAD, P), np.float32)
    x_pad[:N] = x
    shared = dict(
        w0=np.asarray(W1, np.float32).astype(BF16),
        w1=np.asarray(W2, np.float32).astype(BF16),
        w2=np.asarray(W3, np.float32).astype(BF16),
        w3=np.asarray(Wh, np.float32).astype(BF16),
        whead=np.concatenate(
            [np.asarray(Wace, np.float32), np.asarray(Wh2, np.float32)], axis=1
        ).astype(BF16),
        b0=np.asarray(b1, np.float32).reshape(P, 1),
        b1=np.asarray(b2, np.float32).reshape(P, 1),
        b2=np.asarray(b3, np.float32).reshape(P, 1),
        b3=np.asarray(bh, np.float32).reshape(P, 1),
        bhead=np.array(
            [[np.float32(np.asarray(bace).reshape(-1)[0])],
             [np.float32(np.asarray(bh2).reshape(-1)[0])]],
            np.float32,
        ),
    )
    in_maps = []
    for cix in range(NCORES):
        in_maps.append(
            dict(
                x=x_pad[cix * RPC : (cix + 1) * RPC],
                idx=plan["idx"][cix],
                dl=plan["dl"][cix],
                c=plan["c"][cix],
                dinv=plan["dinv"][cix],
                **shared,
            )
        )

    res = bass_utils.run_bass_kernel_spmd(
        nc, in_maps, core_ids=list(range(NCORES)), trace=False
    )

    # host-side unshard: pick target rows from the owning cores
    ace = np.asarray(ace_idx, np.int64)
    h2 = np.asarray(h2_idx, np.int64)
    outs = [r["out"] for r in res.results]
    ace_pred = np.array(
        [outs[i // RPC][0, i % RPC] for i in ace], np.float32
    )
    h2_pred = np.array([outs[i // RPC][1, i % RPC] for i in h2], np.float32)
    result = np.concatenate([ace_pred, h2_pred]).reshape(2 * T, 1).astype(np.float32)
    if _return_exec_info:
        return result, res
    return result
